# revision 7
# baseline (speedup 1.0000x reference)
"""Depthwise 5x5 correlation (stride 1, pad 2) over X[4, 32, 512, 512] fp32,
with a single shared [5, 5] kernel, on 8 Trainium2 NeuronCores.

Strategy (pure data parallel): the 4*32 = 128 images are split 16 per core.
Per image, the 2D conv is decomposed per kernel column j:
    O[h, w] = sum_j C_j[h, w + j - 2],   C_j[h, w] = sum_i K[i, j] X[h+i-2, w]
The H-direction 5-tap conv C_j is one TensorE matmul with a banded-Toeplitz
stationary matrix B_j ([K_rows, M_out], built host-side from the 5x5 kernel).
X is zero-padded along W by 2 on each side host-side ([.., 512, 516]), so the
W-direction shift of C_j is just a shifted rhs read (cols [j, j+512)) and all
five matmuls accumulate the full [M, 512] PSUM range (start=True on the first
zero-fills the bank; fp32r dst patterns must be 2-element aligned, which the
uniform full-width writes satisfy).

H is tiled into 5 row blocks per image (input rows [0,128) [124,252) [248,376)
[372,500) [496,512)) so each block's matmul contraction stays within 128
partitions; the 4-row overlaps replace halo exchanges.

Matmuls run as float32r (single-pass relaxed fp32, 4x faster than strict fp32
on the PE, fp32 PSUM accumulate).
"""

import numpy as np

import concourse.bacc as bacc
import concourse.mybir as mybir
import concourse.tile as tile
from concourse.bass_utils import run_bass_kernel_spmd

F32 = mybir.dt.float32
F32R = mybir.dt.float32r

N_CORES = 8
IMGS_PER_CORE = 16
H = W = 512
WP = W + 4  # W padded by 2 zeros on each side (host-side)
KS = 5

# (t_in, o_out, K_rows, M_out, band_kind)
BLOCKS = [
    (0,   0,   128, 126, "b0"),
    (124, 126, 128, 124, "bm"),
    (248, 250, 128, 124, "bm"),
    (372, 374, 128, 124, "bm"),
    (496, 498, 16,  14,  "b4"),
]
BAND_SHAPES = {"b0": (128, 126, 0), "bm": (128, 124, 2), "b4": (16, 14, 2)}

USE_F32R = True

_CACHE = {}


def build_bands(kern):
    """kern: [5, 5] fp32. Returns dict kind -> [K, 5, M] fp32 ndarray laid out
    partition-major (K on partitions) so the DMA into SBUF is an identity copy.
    B[k, j, m] = kern[k - ot - m + 2, j] where the tap index is in [0, 5)."""
    kern = np.asarray(kern, dtype=np.float32)
    out = {}
    for kind, (K, M, ot) in BAND_SHAPES.items():
        B = np.zeros((K, 5, M), dtype=np.float32)
        k_idx = np.arange(K)[:, None]
        m_idx = np.arange(M)[None, :]
        tap = k_idx - ot - m_idx + 2  # [K, M]
        valid = (tap >= 0) & (tap < KS)
        kk, mm = np.nonzero(valid)
        for j in range(KS):
            B[kk, j, mm] = kern[tap[kk, mm], j]
        out[kind] = B
    return out


def build_nc():
    # float32r end-to-end on the matmul operand path (DRAM declaration, DMA,
    # SBUF tile, matmul input): walrus' BIR verifier requires the producer of
    # an FP32r matmul operand to emit FP32r. Same 4-byte fp32 bits on the wire.
    mm_dt = F32R if USE_F32R else F32
    nc = bacc.Bacc("TRN2", target_bir_lowering=False, debug=False)

    x = nc.dram_tensor("x", [IMGS_PER_CORE, H, WP], mm_dt, kind="ExternalInput").ap()
    bands_dram = {
        kind: nc.dram_tensor(kind, [K, KS, M], mm_dt, kind="ExternalInput").ap()
        for kind, (K, M, _) in BAND_SHAPES.items()
    }
    y = nc.dram_tensor("y", [IMGS_PER_CORE, H, W], F32, kind="ExternalOutput").ap()

    with tile.TileContext(nc) as tc:
        with (
            tc.tile_pool(name="bands", bufs=1) as bpool,
            tc.tile_pool(name="xin", bufs=6) as xpool,
            tc.tile_pool(name="xin4", bufs=3) as x4pool,
            tc.tile_pool(name="out", bufs=6) as opool,
            tc.tile_pool(name="psum", bufs=7, space="PSUM") as ppool,
        ):
            band_t = {}
            for kind, (K, M, _) in BAND_SHAPES.items():
                bt = bpool.tile([K, KS, M], mm_dt, tag=f"band_{kind}")
                nc.sync.dma_start(out=bt[:], in_=bands_dram[kind][:])
                band_t[kind] = bt

            for img in range(IMGS_PER_CORE):
                xts = []
                for (t, o, K, M, kind) in BLOCKS:
                    if K == 128:
                        xt = xpool.tile([128, WP], mm_dt)
                    else:
                        xt = x4pool.tile([K, WP], mm_dt, tag="x4")
                    nc.sync.dma_start(out=xt[:K, :], in_=x[img, t:t + K, :])
                    xts.append(xt)

                for bi, (t, o, K, M, kind) in enumerate(BLOCKS):
                    xt = xts[bi]
                    bt = band_t[kind]
                    P = ppool.tile([M, W], F32, tag="P")
                    for j in range(KS):
                        nc.tensor.matmul(
                            P[:M, :],
                            bt[:K, j, :M],
                            xt[:K, j:j + W],
                            start=(j == 0),
                            stop=(j == KS - 1),
                        )
                    ot = opool.tile([M, W], F32, tag="o")
                    nc.vector.tensor_copy(ot[:M, :], P[:M, :])
                    nc.sync.dma_start(out=y[img, o:o + M, :], in_=ot[:M, :])

    nc.compile()
    return nc


def kernel(X, kernel, stride, padding):
    assert int(stride) == 1 and int(padding) == 2
    X = np.asarray(X, dtype=np.float32)
    B, C, HH, WW = X.shape
    assert (B * C, HH, WW) == (N_CORES * IMGS_PER_CORE, H, W)

    if "nc" not in _CACHE:
        _CACHE["nc"] = build_nc()
    nc = _CACHE["nc"]

    bands = build_bands(kernel)
    Xp = np.zeros((N_CORES, IMGS_PER_CORE, H, WP), dtype=np.float32)
    Xp[:, :, :, 2:2 + W] = X.reshape(N_CORES, IMGS_PER_CORE, H, W)
    in_maps = [
        {"x": Xp[c], "b0": bands["b0"], "bm": bands["bm"], "b4": bands["b4"]}
        for c in range(N_CORES)
    ]
    res = run_bass_kernel_spmd(
        nc, in_maps, core_ids=list(range(N_CORES)), **_CACHE.get("run_kwargs", {})
    )
    _CACHE["last_results"] = res
    out = np.stack([res.results[c]["y"] for c in range(N_CORES)], axis=0)
    return out.reshape(B, C, HH, WW).astype(np.float32)


# revision 9
# speedup vs baseline: 1.3250x; 1.3250x over previous
"""Depthwise 5x5 correlation (stride 1, pad 2) over X[4, 32, 512, 512] fp32,
with a single shared [5, 5] kernel, on 8 Trainium2 NeuronCores.

Strategy (pure data parallel): the 4*32 = 128 images are split 16 per core.
Per image, the 2D conv is decomposed per kernel column j:
    O[h, w] = sum_j C_j[h, w + j - 2],   C_j[h, w] = sum_i K[i, j] X[h+i-2, w]
The H-direction 5-tap conv C_j is one TensorE matmul with a banded-Toeplitz
stationary matrix B_j ([K_rows, M_out], built host-side from the 5x5 kernel).
X is zero-padded along W by 2 on each side host-side ([.., 512, 516]), so the
W-direction shift of C_j is just a shifted rhs read (cols [j, j+512)) and all
five matmuls accumulate the full [M, 512] PSUM range (start=True on the first
zero-fills the bank; fp32r dst patterns must be 2-element aligned, which the
uniform full-width writes satisfy).

H is tiled into 5 row blocks per image (input rows [0,128) [124,252) [248,376)
[372,500) [496,512)) so each block's matmul contraction stays within 128
partitions; the 4-row overlaps replace halo exchanges.

Matmuls run as float32r (single-pass relaxed fp32, 4x faster than strict fp32
on the PE, fp32 PSUM accumulate).
"""

import numpy as np

import concourse.bacc as bacc
import concourse.mybir as mybir
import concourse.tile as tile
from concourse.bass_utils import run_bass_kernel_spmd

F32 = mybir.dt.float32
F32R = mybir.dt.float32r

N_CORES = 8
IMGS_PER_CORE = 16
H = W = 512
WP = W + 4  # W padded by 2 zeros on each side (host-side)
KS = 5

# (t_in, o_out, K_rows, M_out, band_kind)
BLOCKS = [
    (0,   0,   128, 126, "b0"),
    (124, 126, 128, 124, "bm"),
    (248, 250, 128, 124, "bm"),
    (372, 374, 128, 124, "bm"),
    (496, 498, 16,  14,  "b4"),
]
BAND_SHAPES = {"b0": (128, 126, 0), "bm": (128, 124, 2), "b4": (16, 14, 2)}

USE_F32R = True

_CACHE = {}


def build_bands(kern):
    """kern: [5, 5] fp32. Returns dict kind -> [K, 5, M] fp32 ndarray laid out
    partition-major (K on partitions) so the DMA into SBUF is an identity copy.
    B[k, j, m] = kern[k - ot - m + 2, j] where the tap index is in [0, 5)."""
    kern = np.asarray(kern, dtype=np.float32)
    out = {}
    for kind, (K, M, ot) in BAND_SHAPES.items():
        B = np.zeros((K, 5, M), dtype=np.float32)
        k_idx = np.arange(K)[:, None]
        m_idx = np.arange(M)[None, :]
        tap = k_idx - ot - m_idx + 2  # [K, M]
        valid = (tap >= 0) & (tap < KS)
        kk, mm = np.nonzero(valid)
        for j in range(KS):
            B[kk, j, mm] = kern[tap[kk, mm], j]
        out[kind] = B
    return out


def build_nc():
    # float32r end-to-end on the matmul operand path (DRAM declaration, DMA,
    # SBUF tile, matmul input): walrus' BIR verifier requires the producer of
    # an FP32r matmul operand to emit FP32r. Same 4-byte fp32 bits on the wire.
    mm_dt = F32R if USE_F32R else F32
    nc = bacc.Bacc("TRN2", target_bir_lowering=False, debug=False)

    x = nc.dram_tensor("x", [IMGS_PER_CORE, H, WP], mm_dt, kind="ExternalInput").ap()
    bands_dram = {
        kind: nc.dram_tensor(kind, [K, KS, M], mm_dt, kind="ExternalInput").ap()
        for kind, (K, M, _) in BAND_SHAPES.items()
    }
    y = nc.dram_tensor("y", [IMGS_PER_CORE, H, W], F32, kind="ExternalOutput").ap()

    with tile.TileContext(nc) as tc:
        with (
            tc.tile_pool(name="bands", bufs=1) as bpool,
            tc.tile_pool(name="xin", bufs=6) as xpool,
            tc.tile_pool(name="xin4", bufs=3) as x4pool,
            tc.tile_pool(name="out", bufs=6) as opool,
            tc.tile_pool(name="psum", bufs=7, space="PSUM") as ppool,
        ):
            band_t = {}
            for kind, (K, M, _) in BAND_SHAPES.items():
                bt = bpool.tile([K, KS, M], mm_dt, tag=f"band_{kind}")
                nc.sync.dma_start(out=bt[:], in_=bands_dram[kind][:])
                band_t[kind] = bt

            # Two HWDGE rings (SP + ACT): alternate issue engine per DMA so
            # descriptor packets spread across more SDMA engine slots.
            dma_engines = [nc.sync, nc.scalar]
            n_dma = 0

            for img in range(IMGS_PER_CORE):
                xts = []
                for (t, o, K, M, kind) in BLOCKS:
                    if K == 128:
                        xt = xpool.tile([128, WP], mm_dt)
                    else:
                        xt = x4pool.tile([K, WP], mm_dt, tag="x4")
                    dma_engines[n_dma % 2].dma_start(
                        out=xt[:K, :], in_=x[img, t:t + K, :]
                    )
                    n_dma += 1
                    xts.append(xt)

                for bi, (t, o, K, M, kind) in enumerate(BLOCKS):
                    xt = xts[bi]
                    bt = band_t[kind]
                    P = ppool.tile([M, W], F32, tag="P")
                    for j in range(KS):
                        nc.tensor.matmul(
                            P[:M, :],
                            bt[:K, j, :M],
                            xt[:K, j:j + W],
                            start=(j == 0),
                            stop=(j == KS - 1),
                        )
                    ot = opool.tile([M, W], F32, tag="o")
                    nc.vector.tensor_copy(ot[:M, :], P[:M, :])
                    dma_engines[n_dma % 2].dma_start(
                        out=y[img, o:o + M, :], in_=ot[:M, :]
                    )
                    n_dma += 1

    nc.compile()
    return nc


def kernel(X, kernel, stride, padding):
    assert int(stride) == 1 and int(padding) == 2
    X = np.asarray(X, dtype=np.float32)
    B, C, HH, WW = X.shape
    assert (B * C, HH, WW) == (N_CORES * IMGS_PER_CORE, H, W)

    if "nc" not in _CACHE:
        _CACHE["nc"] = build_nc()
    nc = _CACHE["nc"]

    bands = build_bands(kernel)
    Xp = np.zeros((N_CORES, IMGS_PER_CORE, H, WP), dtype=np.float32)
    Xp[:, :, :, 2:2 + W] = X.reshape(N_CORES, IMGS_PER_CORE, H, W)
    in_maps = [
        {"x": Xp[c], "b0": bands["b0"], "bm": bands["bm"], "b4": bands["b4"]}
        for c in range(N_CORES)
    ]
    res = run_bass_kernel_spmd(
        nc, in_maps, core_ids=list(range(N_CORES)), **_CACHE.get("run_kwargs", {})
    )
    _CACHE["last_results"] = res
    out = np.stack([res.results[c]["y"] for c in range(N_CORES)], axis=0)
    return out.reshape(B, C, HH, WW).astype(np.float32)
